# revision 3
# baseline (speedup 1.0000x reference)
"""Bilateral-filter kernel for Trainium2 (Bass/Tile), 8-core data parallel.

Computes, for x[B=32, C=3, Z=64, A=512]:
    out[b, q, k, z, a] = exp(-inv_2theta2[q] * sum_c (x[b,c,z,a] - nbr_k(x)[b,c,z,a])^2)
where nbr_k is the k-th of 14 neighbor shifts in a 3x5 window (center excluded),
zero-padded at the borders, and inv_2theta2 has only 2 distinct values across
the 4 classes.

Sharding: batch 32 -> 8 cores x 4 examples. Per core, 2 "blocks" of 2 examples
stacked on the 128 SBUF partitions (p = b*64 + z). Per block, three padded
input tiles (z-shifted by -1/0/+1) make every neighbor offset a free-dim slice.
"""

import os
import sys

for _p in (
    "/root/.axon_site",
    "/root/.axon_site/_ro/trn_rl_repo",
    "/root/.axon_site/_ro/pypackages",
    "/opt/trn_rl_repo",
):
    if os.path.isdir(_p) and _p not in sys.path:
        sys.path.append(_p)

import numpy as np

import concourse.bacc as bacc
import concourse.bass as bass
import concourse.tile as tile
from concourse import mybir
from concourse.bass_utils import run_bass_kernel_spmd

F32 = mybir.dt.float32
AF = mybir.ActivationFunctionType
ALU = mybir.AluOpType

# Problem constants (hardcoded per contract)
B_FULL, C, Z, A = 32, 3, 64, 512
N_CORES = 8
B_LOCAL = B_FULL // N_CORES          # 4 examples per core
SIZE_Z, SIZE_A = 3, 5
PZ, PA = 1, 2
AP_W = A + 2 * PA                    # 516 padded azimuth width
THETA_R = np.array([0.015, 0.015, 0.01, 0.01], dtype=np.float32)
INV_2T2 = 1.0 / (2.0 * THETA_R**2)   # float32, matches reference rounding
G01 = float(INV_2T2[0])              # classes 0,1
G23 = float(INV_2T2[2])              # classes 2,3

# Reference neighbor order: k enumerates (dz, da), dz-major, center excluded
OFFSETS = [
    (dz, da)
    for dz in range(SIZE_Z)
    for da in range(SIZE_A)
    if not (dz == PZ and da == PA)
]
K = len(OFFSETS)                     # 14

_COMPILED = {}


def _build():
    nc = bacc.Bacc("TRN2", target_bir_lowering=False, debug=False)
    x = nc.dram_tensor("x", [B_LOCAL, C, Z, A], F32, kind="ExternalInput").ap()
    out = nc.dram_tensor(
        "out", [B_LOCAL, 4, K, Z, A], F32, kind="ExternalOutput"
    ).ap()

    with tile.TileContext(nc) as tc:
        with (
            tc.tile_pool(name="inp", bufs=2) as inp_pool,
            tc.tile_pool(name="work", bufs=3) as work_pool,
            tc.tile_pool(name="acc", bufs=4) as acc_pool,
            tc.tile_pool(name="eout", bufs=6) as e_pool,
        ):
            for blk in range(B_LOCAL // 2):
                b0 = 2 * blk
                # One tile per z-shift s in {-1, 0, +1} (dz = s + 1), holding
                # all 3 channels side by side in the free dim, each channel
                # slab AP_W wide with PA zero pad cols on each side.
                # t[p = b*64+z, c*AP_W + PA + a] = x[b0+b, c, z+s, a] (0 outside).
                tM = inp_pool.tile([128, C * AP_W], F32, tag="tM")
                tD = inp_pool.tile([128, C * AP_W], F32, tag="tD")
                tU = inp_pool.tile([128, C * AP_W], F32, tag="tU")

                tM3 = tM[:].rearrange("p (c w) -> p c w", c=C)
                nc.vector.memset(tM3[:, :, 0:PA], 0.0)
                nc.vector.memset(tM3[:, :, PA + A : AP_W], 0.0)
                for c in range(C):
                    nc.sync.dma_start(
                        out=tM[:, c * AP_W + PA : c * AP_W + PA + A],
                        in_=x[b0 : b0 + 2, c, :, :],
                    )
                # z-shifted copies (SBUF->SBUF, full rows so col pads carry
                # over). Engine ops need 32-aligned partition starts, so tU's
                # pad rows (63, 127) are covered by a full-tile memset.
                nc.vector.memset(tU[:], 0.0)
                for b in range(2):
                    p = b * 64
                    nc.vector.memset(tD[p : p + 1, :], 0.0)
                    nc.sync.dma_start(
                        out=tD[p + 1 : p + Z, :], in_=tM[p : p + Z - 1, :]
                    )
                    nc.sync.dma_start(
                        out=tU[p : p + Z - 1, :], in_=tM[p + 1 : p + Z, :]
                    )

                center = tM3[:, :, PA : PA + A]
                tiles3 = {
                    0: tD[:].rearrange("p (c w) -> p c w", c=C),
                    1: tM3,
                    2: tU[:].rearrange("p (c w) -> p c w", c=C),
                }

                # dz == PZ (pure azimuth, needs only tM) first so compute can
                # start before the shifted copies land.
                order = sorted(range(K), key=lambda i: OFFSETS[i][0] != PZ)
                for k in order:
                    dz, da = OFFSETS[k]
                    nbr = tiles3[dz][:, :, da : da + A]
                    d = work_pool.tile([128, C * A], F32, tag="d")
                    nc.vector.tensor_tensor(
                        out=d[:].rearrange("p (c w) -> p c w", c=C),
                        in0=center,
                        in1=nbr,
                        op=ALU.subtract,
                    )
                    sq = work_pool.tile([128, C * A], F32, tag="sq")
                    nc.scalar.activation(sq[:], d[:], AF.Square)
                    t0 = acc_pool.tile([128, A], F32, tag="acc1")
                    nc.vector.tensor_add(t0[:], sq[:, 0:A], sq[:, A : 2 * A])
                    acc = acc_pool.tile([128, A], F32, tag="acc2")
                    nc.vector.tensor_add(acc[:], t0[:], sq[:, 2 * A : 3 * A])

                    for g, qs, tag in (
                        (G01, (0, 1), "e01"),
                        (G23, (2, 3), "e23"),
                    ):
                        e = e_pool.tile([128, A], F32, tag=tag)
                        nc.scalar.activation(e[:], acc[:], AF.Exp, scale=-g)
                        for q in qs:
                            nc.sync.dma_start(
                                out=out[b0 : b0 + 2, q, k, :, :], in_=e[:]
                            )

    nc.compile()
    return nc


def kernel(x: np.ndarray) -> np.ndarray:
    x = np.ascontiguousarray(np.asarray(x, dtype=np.float32))
    assert x.shape == (B_FULL, C, Z, A), x.shape
    if "nc" not in _COMPILED:
        _COMPILED["nc"] = _build()
    nc = _COMPILED["nc"]
    in_maps = [
        {"x": x[i * B_LOCAL : (i + 1) * B_LOCAL]} for i in range(N_CORES)
    ]
    res = run_bass_kernel_spmd(nc, in_maps, list(range(N_CORES)))
    outs = [res.results[i]["out"] for i in range(N_CORES)]
    return np.concatenate(outs, axis=0).astype(np.float32, copy=False)


if __name__ == "__main__":
    xs = np.random.default_rng(0).standard_normal(
        (B_FULL, C, Z, A), dtype=np.float32
    )
    y = kernel(x=xs)
    print(y.shape, y.dtype)
